# revision 10
# baseline (speedup 1.0000x reference)
"""AttentionAggregator Trainium2 kernel (8-core SPMD, data-parallel over nodes).

Math (per node b with neighbors n):
  x_att   = lrelu_.01(x @ W_att);  neib_att = lrelu_.01(neibs @ W_att)
  e[b,n]  = lrelu_.2(x_att[b]@a_x + neib_att[b,n]@a_n)
  att     = softmax_n(e)
  agg[b]  = sum_n att[b,n] * neibs[b,n]
  out     = relu([x@W_fcx, agg@W_fcn])

Key transforms (host-side, exact):
  a_h*lrelu(z_h) summed over h is rewritten as
     sum_{seg1} relu(x . col) - sum_{seg2} relu(x . col)
  over 258 precomputed columns:
     seg1 = [.99*|a_h|*w_h : a_h>=0] + [+.01*(W@a)]
     seg2 = [.99*|a_h|*w_h : a_h<0 ] + [-.01*(W@a)]
  using lrelu(u) = .01u + .99 relu(u), a*lrelu(z)=sign(a)*lrelu(|a|z),
  k*relu(u)=relu(k*u) for k>0, and u = relu(u) - relu(-u).

On-chip per 128-node block: per-tile PE transpose of neibs (fp32, exact),
f32r scores matmul (TF32-class, logits only), relu+accumulate drains split
across ACT/DVE, softmax in a transposed [T,128] layout, attention applied
via per-tile [128,4] block-mask matmuls accumulating agg^T in PSUM (fp32),
then exact fp32 output matmuls.
"""
import warnings
warnings.filterwarnings("ignore")
import numpy as np
from contextlib import ExitStack

import concourse.bass as bass
import concourse.tile as tile
from concourse import bacc, mybir, masks
from concourse.bass_utils import run_bass_kernel_spmd

F32 = mybir.dt.float32
F32R = mybir.dt.float32r
AF = mybir.ActivationFunctionType
ALU = mybir.AluOpType
AX = mybir.AxisListType

N_CORES = 8
B_FULL, NB, D, H, O = 20000, 32, 128, 256, 128
HW6 = 2 * H // 2 + 2  # 258 score columns


def _score_weights(W_att: np.ndarray, a_half: np.ndarray):
    """Build the 258-column relu-pair score weight matrix. Returns (W6, split)."""
    pos = np.where(a_half >= 0)[0]
    neg = np.where(a_half < 0)[0]
    Wabs = W_att * np.abs(a_half)[None, :]
    w_d = (W_att @ a_half).astype(np.float64)
    seg1 = np.concatenate([0.99 * Wabs[:, pos], 0.01 * w_d[:, None]], axis=1)
    seg2 = np.concatenate([0.99 * Wabs[:, neg], -0.01 * w_d[:, None]], axis=1)
    W6 = np.concatenate([seg1, seg2], axis=1).astype(np.float32)
    return W6, seg1.shape[1]


def _blocks(bc):
    out = []
    o = 0
    while o < bc:
        f = min(128, bc - o)
        assert f * NB % 128 == 0
        out.append((o, f))
        o += f
    return out


_PROG_CACHE = {}


def _build_program(bc, split_n, split_x, n_cores=N_CORES, relu_blk=8):
    """Build + compile the SPMD program for bc nodes per core."""
    key = (bc, split_n, split_x, n_cores, relu_blk)
    if key in _PROG_CACHE:
        return _PROG_CACHE[key]

    nc = bacc.Bacc("TRN2", target_bir_lowering=False, debug=False,
                   num_devices=n_cores)

    x_d = nc.dram_tensor("x", [bc, D], F32, kind="ExternalInput").ap()
    ne_d = nc.dram_tensor("ne", [bc * NB, D], F32R, kind="ExternalInput").ap()
    w6n_d = nc.dram_tensor("w6n", [D, HW6], F32, kind="ExternalInput").ap()
    w6x_d = nc.dram_tensor("w6x", [D, HW6], F32, kind="ExternalInput").ap()
    wfcx_d = nc.dram_tensor("wfcx", [D, O], F32, kind="ExternalInput").ap()
    wfcn_d = nc.dram_tensor("wfcn", [D, O], F32, kind="ExternalInput").ap()
    mask_d = nc.dram_tensor("mask", [128, 4], F32, kind="ExternalInput").ap()
    mask4_d = nc.dram_tensor("mask4", [128, 4], F32, kind="ExternalInput").ap()
    psel_d = nc.dram_tensor("psel", [128, 32], F32, kind="ExternalInput").ap()
    cful_d = nc.dram_tensor("cful", [128, HW6], F32, kind="ExternalInput").ap()
    out_d = nc.dram_tensor("out", [bc, 2 * O], F32, kind="ExternalOutput").ap()

    with tile.TileContext(nc) as tc, ExitStack() as ctx:
        consts = ctx.enter_context(tc.tile_pool(name="consts", bufs=1))
        nepool = ctx.enter_context(tc.tile_pool(name="ne", bufs=3))
        ntpool = ctx.enter_context(tc.tile_pool(name="nt", bufs=3))
        sc1 = ctx.enter_context(tc.tile_pool(name="scr_act", bufs=4))
        sc2 = ctx.enter_context(tc.tile_pool(name="scr_dve", bufs=4))
        blkpool = ctx.enter_context(tc.tile_pool(name="blk", bufs=2))
        ps_sc = ctx.enter_context(tc.tile_pool(name="ps_sc", bufs=4, space="PSUM"))
        ps_nt = ctx.enter_context(tc.tile_pool(name="ps_nt", bufs=2, space="PSUM"))
        ps_agg = ctx.enter_context(tc.tile_pool(name="ps_agg", bufs=1, space="PSUM"))
        ps_misc = ctx.enter_context(tc.tile_pool(name="ps_misc", bufs=1, space="PSUM"))

        ident = consts.tile([128, 128], F32)
        masks.make_identity(nc, ident[:])
        w6n32 = consts.tile([D, HW6], F32)
        w6x32 = consts.tile([D, HW6], F32)
        wfcx = consts.tile([D, O], F32)
        wfcn = consts.tile([D, O], F32)
        mask = consts.tile([128, 4], F32)
        mask4 = consts.tile([128, 4], F32)
        psel = consts.tile([128, 32], F32)
        cful = consts.tile([128, HW6], F32)
        for t, d in [(w6n32, w6n_d), (w6x32, w6x_d), (wfcx, wfcx_d),
                     (wfcn, wfcn_d), (mask, mask_d), (mask4, mask4_d),
                     (psel, psel_d), (cful, cful_d)]:
            nc.sync.dma_start(t[:], d)
        w6n = consts.tile([D, HW6], F32R)
        w6x = consts.tile([D, HW6], F32R)
        identr = consts.tile([128, 128], F32R)
        nc.vector.tensor_copy(w6n[:], w6n32[:])
        nc.vector.tensor_copy(w6x[:], w6x32[:])
        nc.vector.tensor_copy(identr[:], ident[:])

        for (boff, F) in _blocks(bc):
            T = F * NB // 128  # score tiles in this block
            rbase = boff * NB

            # ---- neighbor stream in (per-tile DMAs, 64KB contiguous each)
            ne_buf = nepool.tile([128, 32 * D], F32R, tag="ne")
            ne_v = ne_buf[:].rearrange("p (t d) -> p t d", d=D)
            nc.sync.dma_start(
                ne_v[:, :T, :],
                ne_d[rbase: rbase + 128 * T, :].rearrange(
                    "(t p) d -> p t d", p=128))

            # ---- x side
            x_sb = blkpool.tile([128, D], F32, tag="x")
            nc.sync.dma_start(x_sb[:F, :], x_d[boff:boff + F, :])
            xt_ps = ps_misc.tile([128, 258], F32, tag="misc")
            nc.tensor.transpose(xt_ps[:, :F], x_sb[:F, :], ident[:F, :F])
            xt32 = blkpool.tile([D, 128], F32, tag="xt32")
            xtr = blkpool.tile([D, 128], F32R, tag="xtr")
            nc.vector.tensor_copy(xt32[:, :F], xt_ps[:, :F])
            nc.vector.tensor_copy(xtr[:, :F], xt_ps[:, :F])
            xs_ps = ps_misc.tile([128, 258], F32, tag="misc")
            nc.tensor.matmul(xs_ps[:F, :], xtr[:, :F], w6x[:], start=True, stop=True)
            sxacc = blkpool.tile([128, 2], F32, tag="sxacc")
            xscr = sc1.tile([128, HW6], F32, tag="scr_a")
            nc.scalar.activation(xscr[:F, :split_x], xs_ps[:F, :split_x], AF.Relu,
                                 accum_out=sxacc[:F, 0:1])
            nc.scalar.activation(xscr[:F, split_x:HW6], xs_ps[:F, split_x:HW6],
                                 AF.Relu, accum_out=sxacc[:F, 1:2])
            sx = blkpool.tile([128, 1], F32, tag="sx")
            nc.vector.tensor_tensor(sx[:F, :], sxacc[:F, 0:1], sxacc[:F, 1:2],
                                    op=ALU.subtract)
            sx4 = blkpool.tile([128, 4], F32, tag="sx4")
            nc.vector.tensor_scalar(sx4[:F, :], mask4[:F, :], sx[:F, 0:1], None,
                                    op0=ALU.mult)
            sxg_ps = ps_misc.tile([128, 258], F32, tag="misc")
            nc.tensor.matmul(sxg_ps[:T, 0:4], psel[:F, :T], sx4[:F, :],
                             start=True, stop=True)
            sxg = blkpool.tile([32, 4], F32, tag="sxg")
            nc.vector.tensor_copy(sxg[:T, :], sxg_ps[:T, 0:4])

            # ---- per-tile: transpose, scores, relu+accum drains
            spos = blkpool.tile([128, 32], F32, tag="spos")
            sneg = blkpool.tile([128, 32], F32, tag="sneg")
            nc.gpsimd.memset(sneg[:, :T], 0.0)
            for t0 in range(0, T, 2):
                npair = min(2, T - t0)
                nt_ps = ps_nt.tile([128, 256], F32R, tag="nt")
                for k in range(npair):
                    t = t0 + k
                    nc.tensor.transpose(nt_ps[:, 128 * k:128 * (k + 1)],
                                        ne_v[:, t, :], identr[:])
                nt_sb = ntpool.tile([128, 256], F32R, tag="nt")
                if (t0 // 2) % 4 == 3:
                    nc.vector.tensor_copy(nt_sb[:, :128 * npair],
                                          nt_ps[:, :128 * npair])
                else:
                    nc.scalar.copy(nt_sb[:, :128 * npair], nt_ps[:, :128 * npair])
                for k in range(npair):
                    t = t0 + k
                    s_ps = ps_sc.tile([128, HW6], F32, tag="sc")
                    nc.tensor.matmul(s_ps[:], nt_sb[:, 128 * k:128 * (k + 1)],
                                     w6n[:], start=True, stop=True)
                    if t % 10 < 2:
                        scr = sc1.tile([128, HW6], F32, tag="scr_a")
                        nc.scalar.activation(scr[:, :split_n], s_ps[:, :split_n],
                                             AF.Relu, accum_out=spos[:, t:t + 1])
                        nc.scalar.activation(scr[:, split_n:HW6],
                                             s_ps[:, split_n:HW6], AF.Relu,
                                             accum_out=sneg[:, t:t + 1])
                    else:
                        scr = sc2.tile([128, HW6], F32, tag="scr_d")
                        nc.vector.scalar_tensor_tensor(
                            scr[:], s_ps[:], 0.0, cful[:],
                            op0=ALU.max, op1=ALU.mult,
                            accum_out=spos[:, t:t + 1])

            # ---- softmax over neighbors in [T, 128] layout
            s_col = blkpool.tile([128, 32], F32, tag="s_col")
            nc.vector.tensor_tensor(s_col[:, :T], spos[:, :T], sneg[:, :T],
                                    op=ALU.subtract)
            snt_ps = ps_misc.tile([128, 258], F32, tag="misc")
            nc.tensor.transpose(snt_ps[:T, :128], s_col[:, :T], ident[:])
            z = blkpool.tile([32, 128], F32, tag="z")
            nc.vector.tensor_tensor(
                z[:T, :].rearrange("t (j n) -> t j n", n=32),
                snt_ps[:T, :128].rearrange("t (j n) -> t j n", n=32),
                sxg[:T, :].unsqueeze(2).broadcast_to([T, 4, 32]),
                op=ALU.add)
            zl = blkpool.tile([32, 128], F32, tag="zl")
            nc.vector.scalar_tensor_tensor(zl[:T, :], z[:T, :], 0.2, z[:T, :],
                                           op0=ALU.mult, op1=ALU.max)
            ex = blkpool.tile([32, 128], F32, tag="ex")
            nc.scalar.activation(ex[:T, :], zl[:T, :], AF.Exp)
            sums = blkpool.tile([32, 4], F32, tag="sums")
            nc.vector.tensor_reduce(
                sums[:T, :], ex[:T, :].rearrange("t (j n) -> t j n", n=32),
                axis=AX.X, op=ALU.add)
            rec = blkpool.tile([32, 4], F32, tag="rec")
            nc.vector.reciprocal(rec[:T, :], sums[:T, :])
            att = blkpool.tile([32, 128], F32, tag="att")
            nc.vector.tensor_tensor(
                att[:T, :].rearrange("t (j n) -> t j n", n=32),
                ex[:T, :].rearrange("t (j n) -> t j n", n=32),
                rec[:T, :].unsqueeze(2).broadcast_to([T, 4, 32]),
                op=ALU.mult)
            att_ps = ps_misc.tile([128, 258], F32, tag="misc")
            nc.tensor.transpose(att_ps[:, :T], att[:T, :], ident[:T, :T])
            a_all = blkpool.tile([128, 128], F32R, tag="a_all")
            nc.vector.tensor_tensor(
                a_all[:].rearrange("p (t j) -> p t j", j=4)[:, :T, :],
                mask[:].unsqueeze(1).broadcast_to([128, T, 4]),
                att_ps[:, :T].unsqueeze(2).broadcast_to([128, T, 4]),
                op=ALU.mult)

            # ---- attention-weighted aggregation (fp32 exact)
            agg_ps = ps_agg.tile([128, 128], F32, tag="agg")
            a_v = a_all[:].rearrange("p (t j) -> p t j", j=4)
            for t in range(T):
                nc.tensor.matmul(agg_ps[:, 4 * t:4 * (t + 1)], ne_v[:, t, :],
                                 a_v[:, t, :], start=True, stop=True)
            aggt = blkpool.tile([D, 128], F32, tag="aggt")
            nc.vector.tensor_copy(aggt[:, :F], agg_ps[:, :F])

            # ---- output matmuls + relu
            fc_ps = ps_misc.tile([128, 258], F32, tag="misc")
            nc.tensor.matmul(fc_ps[:F, 0:O], xt32[:, :F], wfcx[:],
                             start=True, stop=True)
            nc.tensor.matmul(fc_ps[:F, O:2 * O], aggt[:, :F], wfcn[:],
                             start=True, stop=True)
            out_sb = blkpool.tile([128, 2 * O], F32, tag="out")
            nc.vector.tensor_scalar(out_sb[:F, :], fc_ps[:F, :2 * O], 0.0, None,
                                    op0=ALU.max)
            nc.sync.dma_start(out_d[boff:boff + F, :], out_sb[:F, :])

    nc.compile()
    _PROG_CACHE[key] = nc
    return nc


def kernel(x, neibs, W_att, W_fcx, W_fcn, a, n_cores=N_CORES):
    x = np.asarray(x, dtype=np.float32)
    neibs = np.asarray(neibs, dtype=np.float32)
    W_att = np.asarray(W_att, dtype=np.float32)
    W_fcx = np.asarray(W_fcx, dtype=np.float32)
    W_fcn = np.asarray(W_fcn, dtype=np.float32)
    a = np.asarray(a, dtype=np.float32)

    B = x.shape[0]
    bc = B // n_cores
    a_x, a_n = a[:H, 0], a[H:, 0]
    w6x_np, split_x = _score_weights(W_att, a_x)
    w6n_np, split_n = _score_weights(W_att, a_n)
    mask_np = np.equal.outer(np.arange(128) // 32, np.arange(4)).astype(np.float32)
    mask4_np = np.equal.outer(np.arange(128) % 4, np.arange(4)).astype(np.float32)
    psel_np = np.equal.outer(np.arange(128) // 4, np.arange(32)).astype(np.float32)

    nc = _build_program(bc, split_n, split_x, n_cores)

    cvec = np.concatenate([np.ones(split_n), -np.ones(HW6 - split_n)]).astype(np.float32)
    cful_np = np.repeat(cvec[None, :], 128, axis=0)
    shared = {"w6n": w6n_np, "w6x": w6x_np, "wfcx": W_fcx, "wfcn": W_fcn,
              "mask": mask_np, "mask4": mask4_np, "psel": psel_np, "cful": cful_np}
    in_maps = []
    for c in range(n_cores):
        in_maps.append({
            "x": x[c * bc:(c + 1) * bc],
            "ne": neibs[c * bc * NB:(c + 1) * bc * NB],
            **shared,
        })
    res = run_bass_kernel_spmd(nc, in_maps, core_ids=list(range(n_cores)))
    return np.concatenate([res.results[c]["out"] for c in range(n_cores)], axis=0)


# revision 11
# speedup vs baseline: 1.2406x; 1.2406x over previous
"""AttentionAggregator Trainium2 kernel (8-core SPMD, data-parallel over nodes).

Math (per node b with neighbors n):
  x_att   = lrelu_.01(x @ W_att);  neib_att = lrelu_.01(neibs @ W_att)
  e[b,n]  = lrelu_.2(x_att[b]@a_x + neib_att[b,n]@a_n)
  att     = softmax_n(e)
  agg[b]  = sum_n att[b,n] * neibs[b,n]
  out     = relu([x@W_fcx, agg@W_fcn])

Key transforms (host-side, exact):
  a_h*lrelu(z_h) summed over h is rewritten as
     sum_{seg1} relu(x . col) - sum_{seg2} relu(x . col)
  over 258 precomputed columns:
     seg1 = [.99*|a_h|*w_h : a_h>=0] + [+.01*(W@a)]
     seg2 = [.99*|a_h|*w_h : a_h<0 ] + [-.01*(W@a)]
  using lrelu(u) = .01u + .99 relu(u), a*lrelu(z)=sign(a)*lrelu(|a|z),
  k*relu(u)=relu(k*u) for k>0, and u = relu(u) - relu(-u).

On-chip per 128-node block: per-tile PE transpose of neibs (fp32, exact),
f32r scores matmul (TF32-class, logits only), relu+accumulate drains split
across ACT/DVE, softmax in a transposed [T,128] layout, attention applied
via per-tile [128,4] block-mask matmuls accumulating agg^T in PSUM (fp32),
then exact fp32 output matmuls.
"""
import warnings
warnings.filterwarnings("ignore")
import numpy as np
from contextlib import ExitStack

import concourse.bass as bass
import concourse.tile as tile
from concourse import bacc, mybir, masks
from concourse.bass_utils import run_bass_kernel_spmd

F32 = mybir.dt.float32
F32R = mybir.dt.float32r
AF = mybir.ActivationFunctionType
ALU = mybir.AluOpType
AX = mybir.AxisListType

N_CORES = 8
B_FULL, NB, D, H, O = 20000, 32, 128, 256, 128
HW6 = 2 * H // 2 + 2  # 258 score columns


def _score_weights(W_att: np.ndarray, a_half: np.ndarray):
    """Build the 258-column relu-pair score weight matrix. Returns (W6, split)."""
    pos = np.where(a_half >= 0)[0]
    neg = np.where(a_half < 0)[0]
    Wabs = W_att * np.abs(a_half)[None, :]
    w_d = (W_att @ a_half).astype(np.float64)
    seg1 = np.concatenate([0.99 * Wabs[:, pos], 0.01 * w_d[:, None]], axis=1)
    seg2 = np.concatenate([0.99 * Wabs[:, neg], -0.01 * w_d[:, None]], axis=1)
    W6 = np.concatenate([seg1, seg2], axis=1).astype(np.float32)
    return W6, seg1.shape[1]


def _blocks(bc):
    out = []
    o = 0
    while o < bc:
        f = min(128, bc - o)
        assert f * NB % 128 == 0
        out.append((o, f))
        o += f
    return out


_PROG_CACHE = {}


def _build_program(bc, split_n, split_x, n_cores=N_CORES, relu_blk=8):
    """Build + compile the SPMD program for bc nodes per core."""
    key = (bc, split_n, split_x, n_cores, relu_blk)
    if key in _PROG_CACHE:
        return _PROG_CACHE[key]

    nc = bacc.Bacc("TRN2", target_bir_lowering=False, debug=False,
                   num_devices=n_cores)

    x_d = nc.dram_tensor("x", [bc, D], F32, kind="ExternalInput").ap()
    ne_d = nc.dram_tensor("ne", [bc * NB, D], F32R, kind="ExternalInput").ap()
    w6n_d = nc.dram_tensor("w6n", [D, HW6], F32, kind="ExternalInput").ap()
    w6x_d = nc.dram_tensor("w6x", [D, HW6], F32, kind="ExternalInput").ap()
    wfcx_d = nc.dram_tensor("wfcx", [D, O], F32, kind="ExternalInput").ap()
    wfcn_d = nc.dram_tensor("wfcn", [D, O], F32, kind="ExternalInput").ap()
    mask_d = nc.dram_tensor("mask", [128, 4], F32, kind="ExternalInput").ap()
    mask4_d = nc.dram_tensor("mask4", [128, 4], F32, kind="ExternalInput").ap()
    psel_d = nc.dram_tensor("psel", [128, 32], F32, kind="ExternalInput").ap()
    cful_d = nc.dram_tensor("cful", [128, HW6], F32, kind="ExternalInput").ap()
    out_d = nc.dram_tensor("out", [bc, 2 * O], F32, kind="ExternalOutput").ap()

    with tile.TileContext(nc) as tc, ExitStack() as ctx:
        consts = ctx.enter_context(tc.tile_pool(name="consts", bufs=1))
        nepool = ctx.enter_context(tc.tile_pool(name="ne", bufs=3))
        ntpool = ctx.enter_context(tc.tile_pool(name="nt", bufs=3))
        sc1 = ctx.enter_context(tc.tile_pool(name="scr_act", bufs=4))
        sc2 = ctx.enter_context(tc.tile_pool(name="scr_dve", bufs=4))
        blkpool = ctx.enter_context(tc.tile_pool(name="blk", bufs=2))
        ps_sc = ctx.enter_context(tc.tile_pool(name="ps_sc", bufs=3, space="PSUM"))
        ps_nt = ctx.enter_context(tc.tile_pool(name="ps_nt", bufs=2, space="PSUM"))
        ps_agg = ctx.enter_context(tc.tile_pool(name="ps_agg", bufs=1, space="PSUM"))
        ps_misc = ctx.enter_context(tc.tile_pool(name="ps_misc", bufs=2, space="PSUM"))

        ident = consts.tile([128, 128], F32)
        masks.make_identity(nc, ident[:])
        w6n32 = consts.tile([D, HW6], F32)
        w6x32 = consts.tile([D, HW6], F32)
        wfcx = consts.tile([D, O], F32)
        wfcn = consts.tile([D, O], F32)
        mask = consts.tile([128, 4], F32)
        mask4 = consts.tile([128, 4], F32)
        psel = consts.tile([128, 32], F32)
        cful = consts.tile([128, HW6], F32)
        for t, d in [(w6n32, w6n_d), (w6x32, w6x_d), (wfcx, wfcx_d),
                     (wfcn, wfcn_d), (mask, mask_d), (mask4, mask4_d),
                     (psel, psel_d), (cful, cful_d)]:
            nc.sync.dma_start(t[:], d)
        w6n = consts.tile([D, HW6], F32R)
        w6x = consts.tile([D, HW6], F32R)
        identr = consts.tile([128, 128], F32R)
        nc.vector.tensor_copy(w6n[:], w6n32[:])
        nc.vector.tensor_copy(w6x[:], w6x32[:])
        nc.vector.tensor_copy(identr[:], ident[:])

        for (boff, F) in _blocks(bc):
            T = F * NB // 128  # score tiles in this block
            rbase = boff * NB

            # ---- neighbor stream in (per-tile DMAs, 64KB contiguous each)
            ne_buf = nepool.tile([128, 32 * D], F32R, tag="ne")
            ne_v = ne_buf[:].rearrange("p (t d) -> p t d", d=D)
            nc.sync.dma_start(
                ne_v[:, :T, :],
                ne_d[rbase: rbase + 128 * T, :].rearrange(
                    "(t p) d -> p t d", p=128))

            # ---- x side
            x_sb = blkpool.tile([128, D], F32, tag="x")
            nc.sync.dma_start(x_sb[:F, :], x_d[boff:boff + F, :])
            xt_ps = ps_misc.tile([128, 258], F32, tag="misc")
            nc.tensor.transpose(xt_ps[:, :F], x_sb[:F, :], ident[:F, :F])
            xt32 = blkpool.tile([D, 128], F32, tag="xt32")
            xtr = blkpool.tile([D, 128], F32R, tag="xtr")
            nc.vector.tensor_copy(xt32[:, :F], xt_ps[:, :F])
            nc.vector.tensor_copy(xtr[:, :F], xt_ps[:, :F])
            xs_ps = ps_misc.tile([128, 258], F32, tag="misc")
            nc.tensor.matmul(xs_ps[:F, :], xtr[:, :F], w6x[:], start=True, stop=True)
            sxacc = blkpool.tile([128, 2], F32, tag="sxacc")
            xscr = sc1.tile([128, HW6], F32, tag="scr_a")
            nc.scalar.activation(xscr[:F, :split_x], xs_ps[:F, :split_x], AF.Relu,
                                 accum_out=sxacc[:F, 0:1])
            nc.scalar.activation(xscr[:F, split_x:HW6], xs_ps[:F, split_x:HW6],
                                 AF.Relu, accum_out=sxacc[:F, 1:2])
            sx = blkpool.tile([128, 1], F32, tag="sx")
            nc.vector.tensor_tensor(sx[:F, :], sxacc[:F, 0:1], sxacc[:F, 1:2],
                                    op=ALU.subtract)
            sx4 = blkpool.tile([128, 4], F32, tag="sx4")
            nc.vector.tensor_scalar(sx4[:F, :], mask4[:F, :], sx[:F, 0:1], None,
                                    op0=ALU.mult)
            sxg_ps = ps_misc.tile([128, 258], F32, tag="misc")
            nc.tensor.matmul(sxg_ps[:T, 0:4], psel[:F, :T], sx4[:F, :],
                             start=True, stop=True)
            sxg = blkpool.tile([32, 4], F32, tag="sxg")
            nc.vector.tensor_copy(sxg[:T, :], sxg_ps[:T, 0:4])

            # ---- per-tile: transpose, scores, relu+accum drains
            spos = blkpool.tile([128, 32], F32, tag="spos")
            sneg = blkpool.tile([128, 32], F32, tag="sneg")
            nc.gpsimd.memset(sneg[:, :T], 0.0)
            for t0 in range(0, T, 2):
                npair = min(2, T - t0)
                nt_ps = ps_nt.tile([128, 256], F32R, tag="nt")
                for k in range(npair):
                    t = t0 + k
                    nc.tensor.transpose(nt_ps[:, 128 * k:128 * (k + 1)],
                                        ne_v[:, t, :], identr[:])
                nt_sb = ntpool.tile([128, 256], F32R, tag="nt")
                if (t0 // 2) % 4 == 3:
                    nc.vector.tensor_copy(nt_sb[:, :128 * npair],
                                          nt_ps[:, :128 * npair])
                else:
                    nc.scalar.copy(nt_sb[:, :128 * npair], nt_ps[:, :128 * npair])
                for k in range(npair):
                    t = t0 + k
                    s_ps = ps_sc.tile([128, HW6], F32, tag="sc")
                    nc.tensor.matmul(s_ps[:], nt_sb[:, 128 * k:128 * (k + 1)],
                                     w6n[:], start=True, stop=True)
                    if t % 10 < 2:
                        scr = sc1.tile([128, HW6], F32, tag="scr_a")
                        nc.scalar.activation(scr[:, :split_n], s_ps[:, :split_n],
                                             AF.Relu, accum_out=spos[:, t:t + 1])
                        nc.scalar.activation(scr[:, split_n:HW6],
                                             s_ps[:, split_n:HW6], AF.Relu,
                                             accum_out=sneg[:, t:t + 1])
                    else:
                        scr = sc2.tile([128, HW6], F32, tag="scr_d")
                        nc.vector.scalar_tensor_tensor(
                            scr[:], s_ps[:], 0.0, cful[:],
                            op0=ALU.max, op1=ALU.mult,
                            accum_out=spos[:, t:t + 1])

            # ---- softmax over neighbors in [T, 128] layout
            s_col = blkpool.tile([128, 32], F32, tag="s_col")
            nc.vector.tensor_tensor(s_col[:, :T], spos[:, :T], sneg[:, :T],
                                    op=ALU.subtract)
            snt_ps = ps_misc.tile([128, 258], F32, tag="misc")
            nc.tensor.transpose(snt_ps[:T, :128], s_col[:, :T], ident[:])
            z = blkpool.tile([32, 128], F32, tag="z")
            nc.vector.tensor_tensor(
                z[:T, :].rearrange("t (j n) -> t j n", n=32),
                snt_ps[:T, :128].rearrange("t (j n) -> t j n", n=32),
                sxg[:T, :].unsqueeze(2).broadcast_to([T, 4, 32]),
                op=ALU.add)
            zl = blkpool.tile([32, 128], F32, tag="zl")
            nc.vector.scalar_tensor_tensor(zl[:T, :], z[:T, :], 0.2, z[:T, :],
                                           op0=ALU.mult, op1=ALU.max)
            ex = blkpool.tile([32, 128], F32, tag="ex")
            nc.scalar.activation(ex[:T, :], zl[:T, :], AF.Exp)
            sums = blkpool.tile([32, 4], F32, tag="sums")
            nc.vector.tensor_reduce(
                sums[:T, :], ex[:T, :].rearrange("t (j n) -> t j n", n=32),
                axis=AX.X, op=ALU.add)
            rec = blkpool.tile([32, 4], F32, tag="rec")
            nc.vector.reciprocal(rec[:T, :], sums[:T, :])
            att = blkpool.tile([32, 128], F32, tag="att")
            nc.vector.tensor_tensor(
                att[:T, :].rearrange("t (j n) -> t j n", n=32),
                ex[:T, :].rearrange("t (j n) -> t j n", n=32),
                rec[:T, :].unsqueeze(2).broadcast_to([T, 4, 32]),
                op=ALU.mult)
            att_ps = ps_misc.tile([128, 258], F32, tag="misc")
            nc.tensor.transpose(att_ps[:, :T], att[:T, :], ident[:T, :T])
            a_all = blkpool.tile([128, 128], F32R, tag="a_all")
            nc.vector.tensor_tensor(
                a_all[:].rearrange("p (t j) -> p t j", j=4)[:, :T, :],
                mask[:].unsqueeze(1).broadcast_to([128, T, 4]),
                att_ps[:, :T].unsqueeze(2).broadcast_to([128, T, 4]),
                op=ALU.mult)

            # ---- attention-weighted aggregation (fp32 exact)
            agg_ps = ps_agg.tile([128, 128], F32, tag="agg")
            a_v = a_all[:].rearrange("p (t j) -> p t j", j=4)
            for t in range(T):
                nc.tensor.matmul(agg_ps[:, 4 * t:4 * (t + 1)], ne_v[:, t, :],
                                 a_v[:, t, :], start=True, stop=True)
            aggt = blkpool.tile([D, 128], F32, tag="aggt")
            nc.vector.tensor_copy(aggt[:, :F], agg_ps[:, :F])

            # ---- output matmuls + relu
            fc_ps = ps_misc.tile([128, 258], F32, tag="misc")
            nc.tensor.matmul(fc_ps[:F, 0:O], xt32[:, :F], wfcx[:],
                             start=True, stop=True)
            nc.tensor.matmul(fc_ps[:F, O:2 * O], aggt[:, :F], wfcn[:],
                             start=True, stop=True)
            out_sb = blkpool.tile([128, 2 * O], F32, tag="out")
            nc.vector.tensor_scalar(out_sb[:F, :], fc_ps[:F, :2 * O], 0.0, None,
                                    op0=ALU.max)
            nc.sync.dma_start(out_d[boff:boff + F, :], out_sb[:F, :])

    nc.compile()
    _PROG_CACHE[key] = nc
    return nc


def kernel(x, neibs, W_att, W_fcx, W_fcn, a, n_cores=N_CORES):
    x = np.asarray(x, dtype=np.float32)
    neibs = np.asarray(neibs, dtype=np.float32)
    W_att = np.asarray(W_att, dtype=np.float32)
    W_fcx = np.asarray(W_fcx, dtype=np.float32)
    W_fcn = np.asarray(W_fcn, dtype=np.float32)
    a = np.asarray(a, dtype=np.float32)

    B = x.shape[0]
    bc = B // n_cores
    a_x, a_n = a[:H, 0], a[H:, 0]
    w6x_np, split_x = _score_weights(W_att, a_x)
    w6n_np, split_n = _score_weights(W_att, a_n)
    mask_np = np.equal.outer(np.arange(128) // 32, np.arange(4)).astype(np.float32)
    mask4_np = np.equal.outer(np.arange(128) % 4, np.arange(4)).astype(np.float32)
    psel_np = np.equal.outer(np.arange(128) // 4, np.arange(32)).astype(np.float32)

    nc = _build_program(bc, split_n, split_x, n_cores)

    cvec = np.concatenate([np.ones(split_n), -np.ones(HW6 - split_n)]).astype(np.float32)
    cful_np = np.repeat(cvec[None, :], 128, axis=0)
    shared = {"w6n": w6n_np, "w6x": w6x_np, "wfcx": W_fcx, "wfcn": W_fcn,
              "mask": mask_np, "mask4": mask4_np, "psel": psel_np, "cful": cful_np}
    in_maps = []
    for c in range(n_cores):
        in_maps.append({
            "x": x[c * bc:(c + 1) * bc],
            "ne": neibs[c * bc * NB:(c + 1) * bc * NB],
            **shared,
        })
    res = run_bass_kernel_spmd(nc, in_maps, core_ids=list(range(n_cores)))
    return np.concatenate([res.results[c]["out"] for c in range(n_cores)], axis=0)
